# revision 7
# baseline (speedup 1.0000x reference)
"""Distributed GAT (IntraSectorGAT) Bass kernel for 8 TRN2 NeuronCores.

Strategy (dst-sharded, matmul scatter):
- Nodes sharded by id across 8 cores (6250 dst nodes each); edges (with self
  loops appended) partitioned by destination core and sorted by dst.
- Each core computes the full augmented projection hext = x @ [W | W@a_src |
  W@a_dst] (the att reductions fold into extra weight columns), storing
  [h | a_src] as bf16 rows for edge gathering plus a separate f32 a_dst table.
- Edge phase: 512-edge superchunks; one indirect DMA gathers 4x128 source
  rows, a second gathers per-edge a_dst. Unnormalized attention weights
  w = exp(leaky_relu(a_src+a_dst)) (softmax max-shift dropped: logits are
  O(5), exactly equivalent algebraically). Messages are scattered into the
  128-row dst block via a one-hot selection matmul that also accumulates the
  softmax denominator as 4 extra columns; final per-block normalize divides
  and adds bias. Padding edges point at a 255 "no node" row so their one-hot
  column is all zero - they contribute nothing.
"""
import sys

if "/opt/trn_rl_repo" not in sys.path:
    sys.path.insert(0, "/opt/trn_rl_repo")

import numpy as np

from concourse import bass, mybir, tile

P = 128
CORES = 8
SUPER = 4  # 128-edge chunks per superchunk


# ---------------------------------------------------------------- walrus fix
def _split_waits(nc, k=1):
    """This walrus build accepts at most one sync wait per instruction; move
    excess waits onto preceding same-engine NoOps (same serial stream)."""
    ctr = 0
    for bb in nc.m.functions[0].blocks:
        il = bb.instructions
        i = 0
        while i < len(il):
            ins = il[i]
            si = ins.sync_info
            nw = len(si.on_wait) if si is not None else 0
            if nw > k:
                waits = list(si.on_wait)
                ins.sync_info = mybir.SyncInfo(
                    on_wait=waits[-k:], on_update=list(si.on_update)
                )
                pre = waits[:-k]
                for j in range(0, len(pre), k):
                    ctr += 1
                    il.insert(
                        i,
                        mybir.InstNoOp(
                            name=f"wsplit_{ctr}",
                            engine=ins.engine,
                            sync_info=mybir.SyncInfo(on_wait=pre[j : j + k], on_update=[]),
                        ),
                    )
                    i += 1
            i += 1


# ---------------------------------------------------------------- host prep
def _prep(x, W, att_src, att_dst, bias, edge_index, n_cores):
    N, IN = x.shape
    H, C = att_src.shape
    HC = H * C
    NPC = N // n_cores  # nodes per core

    W_aug = np.concatenate(
        [
            W.astype(np.float32),
            np.stack([W[:, h * C : (h + 1) * C] @ att_src[h] for h in range(H)], 1),
            np.stack([W[:, h * C : (h + 1) * C] @ att_dst[h] for h in range(H)], 1),
        ],
        axis=1,
    ).astype(np.float32)  # [IN, HC+2H]
    xT = np.ascontiguousarray(x.T.astype(np.float32))  # [IN, N]

    src = np.concatenate([edge_index[0], np.arange(N)]).astype(np.int64)
    dst = np.concatenate([edge_index[1], np.arange(N)]).astype(np.int64)

    core_of = dst // NPC
    nblk = (NPC + P - 1) // P
    # per (core, block) edge lists
    counts = np.zeros((n_cores, nblk), np.int64)
    order = np.lexsort((dst,))
    src, dst = src[order], dst[order]
    coreids = dst // NPC
    blkids = (dst % NPC) // P
    for c in range(n_cores):
        m = coreids == c
        bc = np.bincount(blkids[m], minlength=nblk)
        counts[c] = bc
    kmax = np.maximum((counts.max(axis=0) + P - 1) // P, 1)  # chunks per block
    nchunk = int(kmax.sum())
    pad_extra = (-nchunk) % SUPER
    kmax[-1] += pad_extra
    nchunk += pad_extra
    ns = nchunk // SUPER

    src_idx = np.zeros((n_cores, nchunk * P), np.int32)
    dst_idx = np.zeros((n_cores, nchunk * P), np.int32)
    dbv = np.full((n_cores, nchunk * P), 255.0, np.float32)

    sched = []  # (block, start, stop) per chunk — uniform across cores
    for b in range(nblk):
        for k in range(kmax[b]):
            sched.append((b, k == 0, k == kmax[b] - 1))

    starts = np.concatenate([[0], np.cumsum(kmax) * P])  # slot offset per block
    for c in range(n_cores):
        m = coreids == c
        s_c, d_c, b_c = src[m], dst[m], blkids[m]
        for b in range(nblk):
            mb = b_c == b
            cnt = int(mb.sum())
            off = int(starts[b])
            src_idx[c, off : off + cnt] = s_c[mb]
            dst_idx[c, off : off + cnt] = d_c[mb]
            dbv[c, off : off + cnt] = (d_c[mb] % NPC) % P

    # edge e = chunk*128 + p ; gather layout wants [superchunk, p, j]
    def relay(a):
        return np.ascontiguousarray(
            a.reshape(n_cores, ns, SUPER, P).transpose(0, 1, 3, 2)
        )

    return dict(
        W_aug=W_aug,
        xT=xT,
        src_idx=relay(src_idx),
        dst_idx=relay(dst_idx),
        dbv=relay(dbv),
        sched=sched,
        ns=ns,
        nblk=nblk,
        NPC=NPC,
        N=N,
        H=H,
        C=C,
        HC=HC,
        IN=IN,
        iota=np.tile(np.arange(P, dtype=np.float32), (P, 1)),
        bias_row=np.tile(bias.astype(np.float32).reshape(1, HC), (P, 1)),
    )


# ---------------------------------------------------------------- device build
def _build(meta, dbg=False):
    N, IN, H, C, HC = meta["N"], meta["IN"], meta["H"], meta["C"], meta["HC"]
    NPC, nblk, ns = meta["NPC"], meta["nblk"], meta["ns"]
    AUG = HC + 2 * H
    GW = HC + H  # gathered row width (h | a_src)
    f32, bf16, i32 = mybir.dt.float32, mybir.dt.bfloat16, mybir.dt.int32

    nc = bass.Bass()
    xT = nc.declare_dram_parameter("xT", [IN, N], f32, isOutput=False)
    W_aug = nc.declare_dram_parameter("W_aug", [IN, AUG], f32, isOutput=False)
    src_idx = nc.declare_dram_parameter("src_idx", [ns, P, SUPER], i32, isOutput=False)
    dst_idx = nc.declare_dram_parameter("dst_idx", [ns, P, SUPER], i32, isOutput=False)
    dbv = nc.declare_dram_parameter("dbv", [ns, P, SUPER], f32, isOutput=False)
    iota = nc.declare_dram_parameter("iota", [P, P], f32, isOutput=False)
    bias_row = nc.declare_dram_parameter("bias_row", [P, HC], f32, isOutput=False)
    out = nc.declare_dram_parameter("out", [NPC, HC], f32, isOutput=True)

    if dbg:
        hext = nc.declare_dram_parameter("hext", [N, GW], bf16, isOutput=True)
        adst = nc.declare_dram_parameter("adst", [N, H], f32, isOutput=True)
        g4d = nc.declare_dram_parameter("g4d", [P, SUPER * GW], bf16, isOutput=True)
        a4d = nc.declare_dram_parameter("a4d", [P, SUPER * H], f32, isOutput=True)
        s01d = nc.declare_dram_parameter("s01d", [P, P], bf16, isOutput=True)
        msgd = nc.declare_dram_parameter("msgd", [P, SUPER * (HC + H)], bf16, isOutput=True)
        wexd = nc.declare_dram_parameter("wexd", [P, SUPER * H], f32, isOutput=True)
    else:
        hext = nc.dram_tensor("hext", [N, GW], bf16)
        adst = nc.dram_tensor("adst", [N, H], f32)

    sched = meta["sched"]
    with tile.TileContext(nc) as tc:
        with (
            tc.tile_pool(name="const", bufs=1) as cpool,
            tc.tile_pool(name="sbuf", bufs=4) as pool,
            tc.tile_pool(name="psA", bufs=4, space="PSUM") as psA,
            tc.tile_pool(name="psB", bufs=4, space="PSUM") as psB,
        ):
            wt = cpool.tile([IN, AUG], f32)
            nc.sync.dma_start(out=wt[:], in_=W_aug[:])
            iot = cpool.tile([P, P], f32)
            nc.sync.dma_start(out=iot[:], in_=iota[:])
            bt = cpool.tile([P, HC], f32)
            nc.sync.dma_start(out=bt[:], in_=bias_row[:])

            # ---- Phase A: hext = x @ W_aug for ALL nodes (needed for gathers)
            nA = (N + P - 1) // P
            for b in range(nA):
                base = b * P
                nb = min(P, N - base)
                xt = pool.tile([IN, P], f32, tag="xt")
                nc.sync.dma_start(out=xt[:, :nb], in_=xT[:, base : base + nb])
                ps = psA.tile([P, AUG], f32, tag="psA")
                nc.tensor.matmul(
                    out=ps[:nb, :], lhsT=xt[:, :nb], rhs=wt[:], start=True, stop=True
                )
                hb = pool.tile([P, GW], bf16, tag="hb")
                nc.scalar.activation(
                    out=hb[:nb, :], in_=ps[:nb, :GW], func=mybir.ActivationFunctionType.Copy
                )
                ab = pool.tile([P, H], f32, tag="ab")
                nc.vector.tensor_copy(out=ab[:nb, :], in_=ps[:nb, GW : GW + H])
                nc.sync.dma_start(out=hext[base : base + nb, :], in_=hb[:nb, :])
                nc.sync.dma_start(out=adst[base : base + nb, :], in_=ab[:nb, :])

            # ---- Phase B: edge superchunks
            core_base = 0  # dst block rows are per-core local: out rows [b*128 ...]
            ps_cur = None
            for s in range(ns):
                si = pool.tile([P, SUPER], i32, tag="si")
                nc.sync.dma_start(out=si[:], in_=src_idx[s])
                di = pool.tile([P, SUPER], i32, tag="di")
                nc.sync.dma_start(out=di[:], in_=dst_idx[s])
                dbt = pool.tile([P, SUPER], f32, tag="dbt")
                nc.sync.dma_start(out=dbt[:], in_=dbv[s])

                G4 = pool.tile([P, SUPER, GW], bf16, tag="G4")
                A4 = pool.tile([P, SUPER, H], f32, tag="A4")
                for j in range(SUPER):
                    nc.gpsimd.indirect_dma_start(
                        out=G4[:, j, :],
                        out_offset=None,
                        in_=hext[:],
                        in_offset=bass.IndirectOffsetOnAxis(ap=si[:, j : j + 1], axis=0),
                    )
                    nc.gpsimd.indirect_dma_start(
                        out=A4[:, j, :],
                        out_offset=None,
                        in_=adst[:],
                        in_offset=bass.IndirectOffsetOnAxis(ap=di[:, j : j + 1], axis=0),
                    )

                logit = pool.tile([P, SUPER, H], f32, tag="logit")
                nc.vector.tensor_add(out=logit[:], in0=G4[:, :, HC : HC + H], in1=A4[:])
                wlr = pool.tile([P, SUPER, H], f32, tag="wlr")
                nc.vector.tensor_scalar_mul(wlr[:], in0=logit[:], scalar1=0.2)
                nc.vector.tensor_tensor(
                    out=wlr[:], in0=wlr[:], in1=logit[:], op=mybir.AluOpType.max
                )
                wex = pool.tile([P, SUPER, H], f32, tag="wex")
                nc.scalar.activation(
                    out=wex[:], in_=wlr[:], func=mybir.ActivationFunctionType.Exp
                )
                msg = pool.tile([P, SUPER, HC + H], bf16, tag="msg")
                nc.vector.tensor_copy(out=msg[:, :, HC : HC + H], in_=wex[:])

                if dbg and s == 0:
                    nc.sync.dma_start(out=g4d[:], in_=G4[:].rearrange("p s g -> p (s g)"))
                    nc.sync.dma_start(out=a4d[:], in_=A4[:].rearrange("p s g -> p (s g)"))
                    nc.sync.dma_start(out=wexd[:], in_=wex[:].rearrange("p s g -> p (s g)"))
                for j in range(SUPER):
                    cidx = s * SUPER + j
                    blk, is_start, is_stop = sched[cidx]
                    nc.vector.tensor_mul(
                        out=msg[:, j, :HC].rearrange("p (h c) -> p h c", h=H),
                        in0=G4[:, j, :HC].rearrange("p (h c) -> p h c", h=H),
                        in1=wex[:, j, :, None].to_broadcast([P, H, C]),
                    )
                    s01 = pool.tile([P, P], bf16, tag="s01")
                    nc.vector.tensor_tensor(
                        out=s01[:],
                        in0=dbt[:, j, None].to_broadcast([P, P]),
                        in1=iot[:],
                        op=mybir.AluOpType.is_equal,
                    )
                    if dbg and s == 0 and j == 0:
                        nc.sync.dma_start(out=s01d[:], in_=s01[:])
                        nc.sync.dma_start(out=msgd[:], in_=msg[:].rearrange("p s g -> p (s g)"))
                    if is_start:
                        ps_cur = psB.tile([P, HC + H], f32, tag="psB")
                    nc.tensor.matmul(
                        out=ps_cur[:],
                        lhsT=s01[:],
                        rhs=msg[:, j, :],
                        start=is_start,
                        stop=is_stop,
                        skip_group_check=True,
                    )
                    if is_stop:
                        base = blk * P
                        nb = min(P, NPC - base)
                        rcp = pool.tile([P, H], f32, tag="rcp")
                        nc.vector.reciprocal(out=rcp[:nb], in_=ps_cur[:nb, HC : HC + H])
                        ob = pool.tile([P, HC], f32, tag="ob")
                        nc.vector.tensor_mul(
                            out=ob[:nb].rearrange("p (h c) -> p h c", h=H),
                            in0=ps_cur[:nb, :HC].rearrange("p (h c) -> p h c", h=H),
                            in1=rcp[:nb, :, None].to_broadcast([nb, H, C]),
                        )
                        nc.vector.tensor_add(
                            out=ob[:nb], in0=ob[:nb], in1=bt[:nb, :]
                        )
                        nc.sync.dma_start(out=out[base : base + nb, :], in_=ob[:nb])

    _split_waits(nc, 1)
    return nc


# ---------------------------------------------------------------- entry point
def kernel(x, W, att_src, att_dst, bias, edge_index, _n_cores=CORES, _trace=[False], _dbg=False):
    from concourse.bass_utils import run_bass_kernel_spmd

    x = np.asarray(x)
    meta = _prep(
        np.asarray(x), np.asarray(W), np.asarray(att_src), np.asarray(att_dst),
        np.asarray(bias), np.asarray(edge_index), _n_cores,
    )
    nc = _build(meta, dbg=_dbg)
    common = dict(
        xT=meta["xT"], W_aug=meta["W_aug"], iota=meta["iota"], bias_row=meta["bias_row"]
    )
    in_maps = [
        dict(
            common,
            src_idx=meta["src_idx"][c],
            dst_idx=meta["dst_idx"][c],
            dbv=meta["dbv"][c],
        )
        for c in range(_n_cores)
    ]
    res = run_bass_kernel_spmd(nc, in_maps, list(range(_n_cores)), trace=_trace[0])
    kernel.last_result = res
    return np.concatenate([res.results[c]["out"] for c in range(_n_cores)], axis=0)


# revision 8
# speedup vs baseline: 1.0358x; 1.0358x over previous
"""Distributed GAT (IntraSectorGAT) Bass kernel for 8 TRN2 NeuronCores.

Strategy (dst-sharded, matmul scatter):
- Nodes sharded by id across 8 cores (6250 dst nodes each); edges (with self
  loops appended) partitioned by destination core and sorted by dst.
- Each core computes the full augmented projection hext = x @ [W | W@a_src |
  W@a_dst] (the att reductions fold into extra weight columns), storing
  [h | a_src] as bf16 rows for edge gathering plus a separate f32 a_dst table.
- Edge phase: 512-edge superchunks; one indirect DMA gathers 4x128 source
  rows, a second gathers per-edge a_dst. Unnormalized attention weights
  w = exp(leaky_relu(a_src+a_dst)) (softmax max-shift dropped: logits are
  O(5), exactly equivalent algebraically). Messages are scattered into the
  128-row dst block via a one-hot selection matmul that also accumulates the
  softmax denominator as 4 extra columns; final per-block normalize divides
  and adds bias. Padding edges point at a 255 "no node" row so their one-hot
  column is all zero - they contribute nothing.
"""
import sys

if "/opt/trn_rl_repo" not in sys.path:
    sys.path.insert(0, "/opt/trn_rl_repo")

import numpy as np

from concourse import bass, mybir, tile

P = 128
CORES = 8
SUPER = 8  # 128-edge chunks per superchunk


# ---------------------------------------------------------------- walrus fix
def _split_waits(nc, k=1):
    """This walrus build accepts at most one sync wait per instruction; move
    excess waits onto preceding same-engine NoOps (same serial stream)."""
    ctr = 0
    for bb in nc.m.functions[0].blocks:
        il = bb.instructions
        i = 0
        while i < len(il):
            ins = il[i]
            si = ins.sync_info
            nw = len(si.on_wait) if si is not None else 0
            if nw > k:
                waits = list(si.on_wait)
                ins.sync_info = mybir.SyncInfo(
                    on_wait=waits[-k:], on_update=list(si.on_update)
                )
                pre = waits[:-k]
                for j in range(0, len(pre), k):
                    ctr += 1
                    il.insert(
                        i,
                        mybir.InstNoOp(
                            name=f"wsplit_{ctr}",
                            engine=ins.engine,
                            sync_info=mybir.SyncInfo(on_wait=pre[j : j + k], on_update=[]),
                        ),
                    )
                    i += 1
            i += 1


# ---------------------------------------------------------------- host prep
def _prep(x, W, att_src, att_dst, bias, edge_index, n_cores):
    N, IN = x.shape
    H, C = att_src.shape
    HC = H * C
    NPC = N // n_cores  # nodes per core

    W_aug = np.concatenate(
        [
            W.astype(np.float32),
            np.stack([W[:, h * C : (h + 1) * C] @ att_src[h] for h in range(H)], 1),
            np.stack([W[:, h * C : (h + 1) * C] @ att_dst[h] for h in range(H)], 1),
        ],
        axis=1,
    ).astype(np.float32)  # [IN, HC+2H]
    xT = np.ascontiguousarray(x.T.astype(np.float32))  # [IN, N]

    src = np.concatenate([edge_index[0], np.arange(N)]).astype(np.int64)
    dst = np.concatenate([edge_index[1], np.arange(N)]).astype(np.int64)

    core_of = dst // NPC
    nblk = (NPC + P - 1) // P
    # per (core, block) edge lists
    counts = np.zeros((n_cores, nblk), np.int64)
    order = np.lexsort((dst,))
    src, dst = src[order], dst[order]
    coreids = dst // NPC
    blkids = (dst % NPC) // P
    for c in range(n_cores):
        m = coreids == c
        bc = np.bincount(blkids[m], minlength=nblk)
        counts[c] = bc
    kmax = np.maximum((counts.max(axis=0) + P - 1) // P, 1)  # chunks per block
    nchunk = int(kmax.sum())
    pad_extra = (-nchunk) % SUPER
    kmax[-1] += pad_extra
    nchunk += pad_extra
    ns = nchunk // SUPER

    src_idx = np.zeros((n_cores, nchunk * P), np.int32)
    dst_idx = np.zeros((n_cores, nchunk * P), np.int32)
    dbv = np.full((n_cores, nchunk * P), 255.0, np.float32)

    sched = []  # (block, start, stop) per chunk — uniform across cores
    for b in range(nblk):
        for k in range(kmax[b]):
            sched.append((b, k == 0, k == kmax[b] - 1))

    starts = np.concatenate([[0], np.cumsum(kmax) * P])  # slot offset per block
    for c in range(n_cores):
        m = coreids == c
        s_c, d_c, b_c = src[m], dst[m], blkids[m]
        for b in range(nblk):
            mb = b_c == b
            cnt = int(mb.sum())
            off = int(starts[b])
            src_idx[c, off : off + cnt] = s_c[mb]
            dst_idx[c, off : off + cnt] = d_c[mb]
            dbv[c, off : off + cnt] = (d_c[mb] % NPC) % P

    # edge e = chunk*128 + p ; gather layout wants [superchunk, p, j]
    def relay(a):
        return np.ascontiguousarray(
            a.reshape(n_cores, ns, SUPER, P).transpose(0, 1, 3, 2)
        )

    return dict(
        W_aug=W_aug,
        xT=xT,
        src_idx=np.concatenate([relay(src_idx), relay(dst_idx)], axis=3),
        dbv=relay(dbv),
        sched=sched,
        ns=ns,
        nblk=nblk,
        NPC=NPC,
        N=N,
        H=H,
        C=C,
        HC=HC,
        IN=IN,
        iota=np.tile(np.arange(P, dtype=np.float32), (P, 1)),
        bias_row=np.tile(bias.astype(np.float32).reshape(1, HC), (P, 1)),
    )


# ---------------------------------------------------------------- device build
def _build(meta, dbg=False):
    N, IN, H, C, HC = meta["N"], meta["IN"], meta["H"], meta["C"], meta["HC"]
    NPC, nblk, ns = meta["NPC"], meta["nblk"], meta["ns"]
    AUG = HC + 2 * H
    GW = HC + H  # gathered row width (h | a_src)
    f32, bf16, i32 = mybir.dt.float32, mybir.dt.bfloat16, mybir.dt.int32

    nc = bass.Bass()
    xT = nc.declare_dram_parameter("xT", [IN, N], f32, isOutput=False)
    W_aug = nc.declare_dram_parameter("W_aug", [IN, AUG], f32, isOutput=False)
    src_idx = nc.declare_dram_parameter("src_idx", [ns, P, 2 * SUPER], i32, isOutput=False)
    dbv = nc.declare_dram_parameter("dbv", [ns, P, SUPER], f32, isOutput=False)
    iota = nc.declare_dram_parameter("iota", [P, P], f32, isOutput=False)
    bias_row = nc.declare_dram_parameter("bias_row", [P, HC], f32, isOutput=False)
    out = nc.declare_dram_parameter("out", [NPC, HC], f32, isOutput=True)

    if dbg:
        hext = nc.declare_dram_parameter("hext", [N, GW], bf16, isOutput=True)
        adst = nc.declare_dram_parameter("adst", [N, H], f32, isOutput=True)
        g4d = nc.declare_dram_parameter("g4d", [P, SUPER * GW], bf16, isOutput=True)
        a4d = nc.declare_dram_parameter("a4d", [P, SUPER * H], f32, isOutput=True)
        s01d = nc.declare_dram_parameter("s01d", [P, P], bf16, isOutput=True)
        msgd = nc.declare_dram_parameter("msgd", [P, SUPER * (HC + H)], bf16, isOutput=True)
        wexd = nc.declare_dram_parameter("wexd", [P, SUPER * H], f32, isOutput=True)
    else:
        hext = nc.dram_tensor("hext", [N, GW], bf16)
        adst = nc.dram_tensor("adst", [N, H], f32)

    sched = meta["sched"]
    with tile.TileContext(nc) as tc:
        with (
            tc.tile_pool(name="const", bufs=1) as cpool,
            tc.tile_pool(name="sbuf", bufs=6) as pool,
            tc.tile_pool(name="psA", bufs=4, space="PSUM") as psA,
            tc.tile_pool(name="psB", bufs=4, space="PSUM") as psB,
        ):
            wt = cpool.tile([IN, AUG], f32)
            nc.sync.dma_start(out=wt[:], in_=W_aug[:])
            iot = cpool.tile([P, P], f32)
            nc.sync.dma_start(out=iot[:], in_=iota[:])
            bt = cpool.tile([P, HC], f32)
            nc.sync.dma_start(out=bt[:], in_=bias_row[:])

            # ---- Phase A: hext = x @ W_aug for ALL nodes (needed for gathers)
            nA = (N + P - 1) // P
            for b in range(nA):
                base = b * P
                nb = min(P, N - base)
                xt = pool.tile([IN, P], f32, tag="xt")
                nc.sync.dma_start(out=xt[:, :nb], in_=xT[:, base : base + nb])
                ps = psA.tile([P, AUG], f32, tag="psA")
                nc.tensor.matmul(
                    out=ps[:nb, :], lhsT=xt[:, :nb], rhs=wt[:], start=True, stop=True
                )
                hb = pool.tile([P, GW], bf16, tag="hb")
                nc.scalar.activation(
                    out=hb[:nb, :], in_=ps[:nb, :GW], func=mybir.ActivationFunctionType.Copy
                )
                ab = pool.tile([P, H], f32, tag="ab")
                nc.vector.tensor_copy(out=ab[:nb, :], in_=ps[:nb, GW : GW + H])
                nc.sync.dma_start(out=hext[base : base + nb, :], in_=hb[:nb, :])
                nc.sync.dma_start(out=adst[base : base + nb, :], in_=ab[:nb, :])

            # ---- Phase B: edge superchunks
            core_base = 0  # dst block rows are per-core local: out rows [b*128 ...]
            ps_cur = None
            for s in range(ns):
                sdi = pool.tile([P, 2 * SUPER], i32, tag="sdi")
                nc.sync.dma_start(out=sdi[:], in_=src_idx[s])
                si = sdi[:, :SUPER]
                di = sdi[:, SUPER:]
                dbt = pool.tile([P, SUPER], f32, tag="dbt")
                nc.sync.dma_start(out=dbt[:], in_=dbv[s])

                G4 = pool.tile([P, SUPER, GW], bf16, tag="G4")
                A4 = pool.tile([P, SUPER, H], f32, tag="A4")
                for j in range(SUPER):
                    nc.gpsimd.indirect_dma_start(
                        out=G4[:, j, :],
                        out_offset=None,
                        in_=hext[:],
                        in_offset=bass.IndirectOffsetOnAxis(ap=si[:, j : j + 1], axis=0),
                    )
                    nc.gpsimd.indirect_dma_start(
                        out=A4[:, j, :],
                        out_offset=None,
                        in_=adst[:],
                        in_offset=bass.IndirectOffsetOnAxis(ap=di[:, j : j + 1], axis=0),
                    )

                logit = pool.tile([P, SUPER, H], f32, tag="logit")
                nc.vector.tensor_add(out=logit[:], in0=G4[:, :, HC : HC + H], in1=A4[:])
                wlr = pool.tile([P, SUPER, H], f32, tag="wlr")
                nc.vector.tensor_scalar_mul(wlr[:], in0=logit[:], scalar1=0.2)
                nc.vector.tensor_tensor(
                    out=wlr[:], in0=wlr[:], in1=logit[:], op=mybir.AluOpType.max
                )
                wex = pool.tile([P, SUPER, H], f32, tag="wex")
                nc.scalar.activation(
                    out=wex[:], in_=wlr[:], func=mybir.ActivationFunctionType.Exp
                )
                msg = pool.tile([P, SUPER, HC + H], bf16, tag="msg")
                nc.vector.tensor_copy(out=msg[:, :, HC : HC + H], in_=wex[:])

                if dbg and s == 0:
                    nc.sync.dma_start(out=g4d[:], in_=G4[:].rearrange("p s g -> p (s g)"))
                    nc.sync.dma_start(out=a4d[:], in_=A4[:].rearrange("p s g -> p (s g)"))
                    nc.sync.dma_start(out=wexd[:], in_=wex[:].rearrange("p s g -> p (s g)"))
                for j in range(SUPER):
                    cidx = s * SUPER + j
                    blk, is_start, is_stop = sched[cidx]
                    nc.vector.tensor_mul(
                        out=msg[:, j, :HC].rearrange("p (h c) -> p h c", h=H),
                        in0=G4[:, j, :HC].rearrange("p (h c) -> p h c", h=H),
                        in1=wex[:, j, :, None].to_broadcast([P, H, C]),
                    )
                    s01 = pool.tile([P, P], bf16, tag="s01")
                    nc.vector.tensor_tensor(
                        out=s01[:],
                        in0=dbt[:, j, None].to_broadcast([P, P]),
                        in1=iot[:],
                        op=mybir.AluOpType.is_equal,
                    )
                    if dbg and s == 0 and j == 0:
                        nc.sync.dma_start(out=s01d[:], in_=s01[:])
                        nc.sync.dma_start(out=msgd[:], in_=msg[:].rearrange("p s g -> p (s g)"))
                    if is_start:
                        ps_cur = psB.tile([P, HC + H], f32, tag="psB")
                    nc.tensor.matmul(
                        out=ps_cur[:],
                        lhsT=s01[:],
                        rhs=msg[:, j, :],
                        start=is_start,
                        stop=is_stop,
                        skip_group_check=True,
                    )
                    if is_stop:
                        base = blk * P
                        nb = min(P, NPC - base)
                        rcp = pool.tile([P, H], f32, tag="rcp")
                        nc.vector.reciprocal(out=rcp[:nb], in_=ps_cur[:nb, HC : HC + H])
                        ob = pool.tile([P, HC], f32, tag="ob")
                        nc.vector.tensor_mul(
                            out=ob[:nb].rearrange("p (h c) -> p h c", h=H),
                            in0=ps_cur[:nb, :HC].rearrange("p (h c) -> p h c", h=H),
                            in1=rcp[:nb, :, None].to_broadcast([nb, H, C]),
                        )
                        nc.vector.tensor_add(
                            out=ob[:nb], in0=ob[:nb], in1=bt[:nb, :]
                        )
                        nc.sync.dma_start(out=out[base : base + nb, :], in_=ob[:nb])

    _split_waits(nc, 1)
    return nc


# ---------------------------------------------------------------- entry point
def kernel(x, W, att_src, att_dst, bias, edge_index, _n_cores=CORES, _trace=[False], _dbg=False):
    from concourse.bass_utils import run_bass_kernel_spmd

    x = np.asarray(x)
    meta = _prep(
        np.asarray(x), np.asarray(W), np.asarray(att_src), np.asarray(att_dst),
        np.asarray(bias), np.asarray(edge_index), _n_cores,
    )
    nc = _build(meta, dbg=_dbg)
    common = dict(
        xT=meta["xT"], W_aug=meta["W_aug"], iota=meta["iota"], bias_row=meta["bias_row"]
    )
    in_maps = [
        dict(
            common,
            src_idx=meta["src_idx"][c],
            dbv=meta["dbv"][c],
        )
        for c in range(_n_cores)
    ]
    res = run_bass_kernel_spmd(nc, in_maps, list(range(_n_cores)), trace=_trace[0])
    kernel.last_result = res
    return np.concatenate([res.results[c]["out"] for c in range(_n_cores)], axis=0)


# revision 11
# speedup vs baseline: 1.5440x; 1.4907x over previous
"""Distributed GAT (IntraSectorGAT) Bass kernel for 8 TRN2 NeuronCores.

Strategy (dst-sharded, matmul scatter):
- Nodes sharded by id across 8 cores (6250 dst nodes each); edges (with self
  loops appended) partitioned by destination core and sorted by dst.
- Each core computes the full augmented projection hext = x @ [W | W@a_src |
  W@a_dst] (the att reductions fold into extra weight columns), storing
  [h | a_src] as bf16 rows for edge gathering plus a separate f32 a_dst table.
- Edge phase: 512-edge superchunks; one indirect DMA gathers 4x128 source
  rows, a second gathers per-edge a_dst. Unnormalized attention weights
  w = exp(leaky_relu(a_src+a_dst)) (softmax max-shift dropped: logits are
  O(5), exactly equivalent algebraically). Messages are scattered into the
  128-row dst block via a one-hot selection matmul that also accumulates the
  softmax denominator as 4 extra columns; final per-block normalize divides
  and adds bias. Padding edges point at a 255 "no node" row so their one-hot
  column is all zero - they contribute nothing.
"""
import sys

if "/opt/trn_rl_repo" not in sys.path:
    sys.path.insert(0, "/opt/trn_rl_repo")

import numpy as np

from concourse import bass, mybir, tile

P = 128
CORES = 8
SUPER = 8  # 128-edge chunks per superchunk


# ---------------------------------------------------------------- walrus fix
def _split_waits(nc, k=1):
    """This walrus build accepts at most one sync wait per instruction; move
    excess waits onto preceding same-engine NoOps (same serial stream)."""
    ctr = 0
    for bb in nc.m.functions[0].blocks:
        il = bb.instructions
        i = 0
        while i < len(il):
            ins = il[i]
            si = ins.sync_info
            nw = len(si.on_wait) if si is not None else 0
            if nw > k:
                waits = list(si.on_wait)
                ins.sync_info = mybir.SyncInfo(
                    on_wait=waits[-k:], on_update=list(si.on_update)
                )
                pre = waits[:-k]
                for j in range(0, len(pre), k):
                    ctr += 1
                    il.insert(
                        i,
                        mybir.InstNoOp(
                            name=f"wsplit_{ctr}",
                            engine=ins.engine,
                            sync_info=mybir.SyncInfo(on_wait=pre[j : j + k], on_update=[]),
                        ),
                    )
                    i += 1
            i += 1


# ---------------------------------------------------------------- host prep
def _prep(x, W, att_src, att_dst, bias, edge_index, n_cores):
    N, IN = x.shape
    H, C = att_src.shape
    HC = H * C
    NPC = N // n_cores  # nodes per core

    W_aug = np.concatenate(
        [
            W.astype(np.float32),
            np.stack([W[:, h * C : (h + 1) * C] @ att_src[h] for h in range(H)], 1),
            np.stack([W[:, h * C : (h + 1) * C] @ att_dst[h] for h in range(H)], 1),
        ],
        axis=1,
    ).astype(np.float32)  # [IN, HC+2H]
    xT = np.ascontiguousarray(x.T.astype(np.float32))  # [IN, N]

    src = np.concatenate([edge_index[0], np.arange(N)]).astype(np.int64)
    dst = np.concatenate([edge_index[1], np.arange(N)]).astype(np.int64)

    core_of = dst // NPC
    nblk = (NPC + P - 1) // P
    # per (core, block) edge lists
    counts = np.zeros((n_cores, nblk), np.int64)
    order = np.lexsort((dst,))
    src, dst = src[order], dst[order]
    coreids = dst // NPC
    blkids = (dst % NPC) // P
    for c in range(n_cores):
        m = coreids == c
        bc = np.bincount(blkids[m], minlength=nblk)
        counts[c] = bc
    kmax = np.maximum((counts.max(axis=0) + P - 1) // P, 1)  # chunks per block
    nchunk = int(kmax.sum())
    pad_extra = (-nchunk) % SUPER
    kmax[-1] += pad_extra
    nchunk += pad_extra
    ns = nchunk // SUPER

    src_idx = np.zeros((n_cores, nchunk * P), np.int32)
    dst_idx = np.zeros((n_cores, nchunk * P), np.int32)
    dbv = np.full((n_cores, nchunk * P), 255.0, np.float32)

    sched = []  # (block, start, stop) per chunk — uniform across cores
    for b in range(nblk):
        for k in range(kmax[b]):
            sched.append((b, k == 0, k == kmax[b] - 1))

    starts = np.concatenate([[0], np.cumsum(kmax) * P])  # slot offset per block
    for c in range(n_cores):
        m = coreids == c
        s_c, d_c, b_c = src[m], dst[m], blkids[m]
        for b in range(nblk):
            mb = b_c == b
            cnt = int(mb.sum())
            off = int(starts[b])
            src_idx[c, off : off + cnt] = s_c[mb]
            dst_idx[c, off : off + cnt] = d_c[mb]
            dbv[c, off : off + cnt] = (d_c[mb] % NPC) % P

    # edge e = chunk*128 + p ; gather layout wants [superchunk, p, j]
    def relay(a):
        return np.ascontiguousarray(
            a.reshape(n_cores, ns, SUPER, P).transpose(0, 1, 3, 2)
        )

    blk_nodes = np.zeros((n_cores, nblk, P), np.int32)
    for c in range(n_cores):
        for b in range(nblk):
            ids = c * NPC + b * P + np.arange(P)
            blk_nodes[c, b] = np.minimum(ids, (c + 1) * NPC - 1)
    return dict(
        W_aug=W_aug,
        xT=xT,
        blk_nodes=blk_nodes,
        src_idx=np.concatenate([relay(src_idx), relay(dst_idx)], axis=3),
        dbv=relay(dbv),
        sched=sched,
        ns=ns,
        nblk=nblk,
        NPC=NPC,
        N=N,
        H=H,
        C=C,
        HC=HC,
        IN=IN,
        iota=np.tile(np.arange(P, dtype=np.float32), (P, 1)),
        bias_row=np.tile(bias.astype(np.float32).reshape(1, HC), (P, 1)),
    )


# ---------------------------------------------------------------- device build
def _build(meta, dbg=False):
    N, IN, H, C, HC = meta["N"], meta["IN"], meta["H"], meta["C"], meta["HC"]
    NPC, nblk, ns = meta["NPC"], meta["nblk"], meta["ns"]
    AUG = HC + 2 * H
    GW = HC + H  # gathered row width (h | a_src)
    f32, bf16, i32 = mybir.dt.float32, mybir.dt.bfloat16, mybir.dt.int32

    nc = bass.Bass()
    xT = nc.declare_dram_parameter("xT", [IN, N], f32, isOutput=False)
    W_aug = nc.declare_dram_parameter("W_aug", [IN, AUG], f32, isOutput=False)
    src_idx = nc.declare_dram_parameter("src_idx", [ns, P, 2 * SUPER], i32, isOutput=False)
    dbv = nc.declare_dram_parameter("dbv", [ns, P, SUPER], f32, isOutput=False)
    iota = nc.declare_dram_parameter("iota", [P, P], f32, isOutput=False)
    blk_nodes = nc.declare_dram_parameter("blk_nodes", [nblk, P], i32, isOutput=False)
    ident = nc.declare_dram_parameter("ident", [P, P], f32, isOutput=False)
    bias_row = nc.declare_dram_parameter("bias_row", [P, HC], f32, isOutput=False)
    out = nc.declare_dram_parameter("out", [NPC, HC], f32, isOutput=True)

    if dbg:
        hext = nc.declare_dram_parameter("hext", [N, GW], bf16, isOutput=True)
        adst = nc.declare_dram_parameter("adst", [N, H], f32, isOutput=True)
        g4d = nc.declare_dram_parameter("g4d", [P, SUPER * GW], bf16, isOutput=True)
        a4d = nc.declare_dram_parameter("a4d", [P, SUPER * H], f32, isOutput=True)
        s01d = nc.declare_dram_parameter("s01d", [P, P], bf16, isOutput=True)
        msgd = nc.declare_dram_parameter("msgd", [P, SUPER * (HC + H)], bf16, isOutput=True)
        wexd = nc.declare_dram_parameter("wexd", [P, SUPER * H], f32, isOutput=True)
    else:
        hext = nc.dram_tensor("hext", [N, GW], bf16)
        adst = nc.dram_tensor("adst", [N, H], f32)

    sched = meta["sched"]
    with tile.TileContext(nc) as tc:
        with (
            tc.tile_pool(name="const", bufs=1) as cpool,
            tc.tile_pool(name="sbuf", bufs=6) as pool,
            tc.tile_pool(name="psA", bufs=2, space="PSUM") as psA,
            tc.tile_pool(name="psB", bufs=3, space="PSUM") as psB,
            tc.tile_pool(name="psT", bufs=2, space="PSUM") as psT,
            tc.tile_pool(name="psD", bufs=1, space="PSUM") as psD,
        ):
            wt = cpool.tile([IN, AUG], f32)
            nc.sync.dma_start(out=wt[:], in_=W_aug[:])
            iot = cpool.tile([P, P], f32)
            nc.sync.dma_start(out=iot[:], in_=iota[:])
            idf = cpool.tile([P, P], f32)
            nc.sync.dma_start(out=idf[:], in_=ident[:])
            idb = cpool.tile([P, P], bf16)
            nc.scalar.activation(out=idb[:], in_=idf[:], func=mybir.ActivationFunctionType.Copy)
            bt = cpool.tile([P, HC], f32)
            nc.sync.dma_start(out=bt[:], in_=bias_row[:])

            # ---- Phase A: hext = x @ W_aug for ALL nodes (needed for gathers)
            nA = (N + P - 1) // P
            for b in range(nA):
                base = b * P
                nb = min(P, N - base)
                xt = pool.tile([IN, P], f32, tag="xt")
                nc.sync.dma_start(out=xt[:, :nb], in_=xT[:, base : base + nb])
                ps = psA.tile([P, AUG], f32, tag="psA")
                nc.tensor.matmul(
                    out=ps[:nb, :], lhsT=xt[:, :nb], rhs=wt[:], start=True, stop=True
                )
                hb = pool.tile([P, GW], bf16, tag="hb")
                nc.scalar.activation(
                    out=hb[:nb, :], in_=ps[:nb, :GW], func=mybir.ActivationFunctionType.Copy
                )
                ab = pool.tile([P, H], f32, tag="ab")
                nc.vector.tensor_copy(out=ab[:nb, :], in_=ps[:nb, GW : GW + H])
                nc.sync.dma_start(out=hext[base : base + nb, :], in_=hb[:nb, :])
                nc.sync.dma_start(out=adst[base : base + nb, :], in_=ab[:nb, :])

            # ---- Phase B: edge superchunks
            core_base = 0  # dst block rows are per-core local: out rows [b*128 ...]
            ps_cur = None
            for s in range(ns):
                sdi = pool.tile([P, 2 * SUPER], i32, tag="sdi")
                nc.sync.dma_start(out=sdi[:], in_=src_idx[s])
                si = sdi[:, :SUPER]
                di = sdi[:, SUPER:]
                dbt = pool.tile([P, SUPER], f32, tag="dbt")
                nc.sync.dma_start(out=dbt[:], in_=dbv[s])

                G4 = pool.tile([P, SUPER, GW], bf16, tag="G4")
                for j in range(SUPER):
                    nc.gpsimd.indirect_dma_start(
                        out=G4[:, j, :],
                        out_offset=None,
                        in_=hext[:],
                        in_offset=bass.IndirectOffsetOnAxis(ap=si[:, j : j + 1], axis=0),
                    )

                wex = pool.tile([P, SUPER, H], f32, tag="wex")
                msg = pool.tile([P, SUPER, HC + H], bf16, tag="msg")

                if dbg and s == 0:
                    nc.sync.dma_start(out=g4d[:], in_=G4[:].rearrange("p s g -> p (s g)"))
                    nc.sync.dma_start(out=a4d[:], in_=A4[:].rearrange("p s g -> p (s g)"))
                    nc.sync.dma_start(out=wexd[:], in_=wex[:].rearrange("p s g -> p (s g)"))
                for j in range(SUPER):
                    cidx = s * SUPER + j
                    blk, is_start, is_stop = sched[cidx]
                    if is_start:
                        bn = pool.tile([P, 1], i32, tag="bn")
                        nc.sync.dma_start(out=bn[:], in_=blk_nodes[blk, :, None])
                        ablk_f = pool.tile([P, H], f32, tag="ablk_f")
                        nc.gpsimd.indirect_dma_start(
                            out=ablk_f[:],
                            out_offset=None,
                            in_=adst[:],
                            in_offset=bass.IndirectOffsetOnAxis(ap=bn[:], axis=0),
                        )
                        ablk = pool.tile([P, H], bf16, tag="ablk")
                        nc.vector.tensor_copy(out=ablk[:], in_=ablk_f[:])
                    s01 = pool.tile([P, P], bf16, tag="s01")
                    nc.vector.tensor_tensor(
                        out=s01[:],
                        in0=dbt[:, j, None].to_broadcast([P, P]),
                        in1=iot[:],
                        op=mybir.AluOpType.is_equal,
                    )
                    s01T_ps = psT.tile([P, P], f32, tag="psT")
                    nc.tensor.matmul(
                        out=s01T_ps[:], lhsT=s01[:], rhs=idb[:], start=True, stop=True,
                        skip_group_check=True,
                    )
                    s01T = pool.tile([P, P], bf16, tag="s01T")
                    nc.scalar.activation(
                        out=s01T[:], in_=s01T_ps[:], func=mybir.ActivationFunctionType.Copy
                    )
                    adg = psD.tile([P, H], f32, tag="psD")
                    nc.tensor.matmul(
                        out=adg[:], lhsT=s01T[:], rhs=ablk[:], start=True, stop=True,
                        skip_group_check=True,
                    )
                    logit = pool.tile([P, H], f32, tag="logit")
                    nc.vector.tensor_add(out=logit[:], in0=G4[:, j, HC : HC + H], in1=adg[:])
                    wlr = pool.tile([P, H], f32, tag="wlr")
                    nc.vector.tensor_scalar_mul(wlr[:], in0=logit[:], scalar1=0.2)
                    nc.vector.tensor_tensor(
                        out=wlr[:], in0=wlr[:], in1=logit[:], op=mybir.AluOpType.max
                    )
                    nc.scalar.activation(
                        out=wex[:, j, :], in_=wlr[:], func=mybir.ActivationFunctionType.Exp
                    )
                    nc.vector.tensor_copy(out=msg[:, j, HC : HC + H], in_=wex[:, j, :])
                    nc.vector.tensor_mul(
                        out=msg[:, j, :HC].rearrange("p (h c) -> p h c", h=H),
                        in0=G4[:, j, :HC].rearrange("p (h c) -> p h c", h=H),
                        in1=wex[:, j, :, None].to_broadcast([P, H, C]),
                    )
                    s01 = pool.tile([P, P], bf16, tag="s01")
                    nc.vector.tensor_tensor(
                        out=s01[:],
                        in0=dbt[:, j, None].to_broadcast([P, P]),
                        in1=iot[:],
                        op=mybir.AluOpType.is_equal,
                    )
                    if dbg and s == 0 and j == 0:
                        nc.sync.dma_start(out=s01d[:], in_=s01[:])
                        nc.sync.dma_start(out=msgd[:], in_=msg[:].rearrange("p s g -> p (s g)"))
                    if is_start:
                        ps_cur = psB.tile([P, HC + H], f32, tag="psB")
                    nc.tensor.matmul(
                        out=ps_cur[:],
                        lhsT=s01[:],
                        rhs=msg[:, j, :],
                        start=is_start,
                        stop=is_stop,
                        skip_group_check=True,
                    )
                    if is_stop:
                        base = blk * P
                        nb = min(P, NPC - base)
                        rcp = pool.tile([P, H], f32, tag="rcp")
                        nc.vector.reciprocal(out=rcp[:nb], in_=ps_cur[:nb, HC : HC + H])
                        ob = pool.tile([P, HC], f32, tag="ob")
                        nc.vector.tensor_mul(
                            out=ob[:nb].rearrange("p (h c) -> p h c", h=H),
                            in0=ps_cur[:nb, :HC].rearrange("p (h c) -> p h c", h=H),
                            in1=rcp[:nb, :, None].to_broadcast([nb, H, C]),
                        )
                        nc.vector.tensor_add(
                            out=ob[:nb], in0=ob[:nb], in1=bt[:nb, :]
                        )
                        nc.sync.dma_start(out=out[base : base + nb, :], in_=ob[:nb])

    _split_waits(nc, 1)
    return nc


# ---------------------------------------------------------------- entry point
def kernel(x, W, att_src, att_dst, bias, edge_index, _n_cores=CORES, _trace=[False], _dbg=False):
    from concourse.bass_utils import run_bass_kernel_spmd

    x = np.asarray(x)
    meta = _prep(
        np.asarray(x), np.asarray(W), np.asarray(att_src), np.asarray(att_dst),
        np.asarray(bias), np.asarray(edge_index), _n_cores,
    )
    nc = _build(meta, dbg=_dbg)
    common = dict(
        xT=meta["xT"], W_aug=meta["W_aug"], iota=meta["iota"], bias_row=meta["bias_row"],
        ident=np.eye(P, dtype=np.float32),
    )
    in_maps = [
        dict(
            common,
            src_idx=meta["src_idx"][c],
            blk_nodes=meta["blk_nodes"][c],
            dbv=meta["dbv"][c],
        )
        for c in range(_n_cores)
    ]
    res = run_bass_kernel_spmd(nc, in_maps, list(range(_n_cores)), trace=_trace[0])
    kernel.last_result = res
    return np.concatenate([res.results[c]["out"] for c in range(_n_cores)], axis=0)
